# revision 8
# baseline (speedup 1.0000x reference)
"""DTW loss kernel for Trainium2 (8 NeuronCores, Bass/Tile) — v2.

reference: C[b,i,j] = ||s1[b,i]-s2[b,j]||^2 ; DTW DP over [512,512]; return
mean_b sqrt(DTW[b,-1,-1]).

Meet-in-the-middle: DTW_end = min_j F[255,j] + min(B[256,j], B[256,j+1]) where
F is the forward DP over rows 0..255 and B the backward DP. Each core: 16
batch elems x 2 directions = 32 virtual batches (vb) of 256-row half-DPs.

v2 design (vs v1's per-superstep psum gather copies):
- Cost rows come from per-vb matmuls: lhsT = u[vb] chunk [18,128],
  rhs = v[vb] [18,512] -> psum [128 rows, 512 cols] f32. 64 matmuls total
  (5x fewer PE cycles than v1's block-diagonal batching, whose cost was
  dominated by 5 K-chunk passes over the 512-wide output).
- Act/DVE convert psum -> fp16 staging [rows, 512].
- Skewed cost buffer csk[q*32+vb, s*W + j] = C[vb, s-2q, q*W+j] in SBUF.
  A single SBUF->SBUF DMA cannot move data across the partition-dim meaning
  (SBUF APs are partition-major), so the reshape takes a DRAM round-trip:
  stage -> dram c_rt[vb, r, j] per vb, then dram -> csk per (band, vb-group)
  with matching flat iteration orders. The DP scan then reads data1 directly
  from csk with zero per-superstep copies (v1 burned the Scalar engine on 3
  gather copies per superstep).
- Wavefront: NQ=4 bands of W=128 cols on 128 partitions = (band q, vb).
  Superstep s: band q scans row s-2q. Carries between bands ride in col 0
  of the `new` tiles (one [32,1] partition-shifted copy per band boundary,
  2 supersteps of slack, on Act/Pool engines).
"""

import numpy as np

B = 128
L1 = 512
L2 = 512
D = 16
N_CORES = 8
PER_CORE = B // N_CORES  # 16
VB = 2 * PER_CORE  # 32
R = L1 // 2  # 256
KAUG = D + 2  # 18
NQ = 4
W = L2 // NQ  # 128
NSS = R + 2 * (NQ - 1)  # 262
GRP = 8  # vb per staging group
NGRP = VB // GRP  # 4
BIG = 1e30

_CACHE = {}


def _emit(tc, u_d, v_d, out_d):
    import concourse.bass as bass  # noqa: F401
    from concourse import mybir

    F32 = mybir.dt.float32
    F16 = mybir.dt.float16
    BF16 = mybir.dt.bfloat16
    Alu = mybir.AluOpType
    nc = tc.nc

    with (
        tc.tile_pool(name="singles", bufs=1) as sp,
        tc.tile_pool(name="psum", bufs=4, space="PSUM") as pp,
        tc.tile_pool(name="dramp", bufs=1, space="DRAM") as dp,
    ):
        c_rt = dp.tile([VB, R, L2], mybir.dt.float16, tag="crt", name="crt")
        u_sb = sp.tile([KAUG, VB * R], BF16, tag="u", name="u")
        v_sb = sp.tile([KAUG, VB * L2], BF16, tag="v", name="v")
        csk = sp.tile([128, NSS * W], F16, tag="csk", name="csk")
        stage = [
            sp.tile([128, L2], F16, tag=f"stg{i}", name=f"stg{i}")
            for i in range(4)
        ]
        new = [sp.tile([128, W + 1], F32, tag=f"new{p}", name=f"new{p}") for p in range(4)]
        mm = [sp.tile([128, W], F32, tag=f"m{p}", name=f"m{p}") for p in range(2)]
        bigm = sp.tile([128, W], F32, tag="bigm", name="bigm")
        init0 = sp.tile([128, 1], F32, tag="init0", name="init0")
        outbuf = sp.tile([32, L2], F32, tag="ob", name="ob")

        # --- prologue ---
        nc.sync.dma_start(out=u_sb, in_=u_d)
        nc.sync.dma_start(out=v_sb, in_=v_d)
        nc.vector.memset(bigm, BIG)
        nc.vector.memset(init0, 0.0)
        for p in range(4):
            nc.vector.memset(new[p][:, 0:1], BIG)
        # csk margins (supersteps where a band has no valid row)
        nc.vector.memset(csk[:, 0 : 2 * (NQ - 1) * W], 0.0)
        nc.vector.memset(csk[:, R * W : NSS * W], 0.0)

        def emit_mm_conv(vb, h, eng):
            pt = pp.tile([128, L2], F32, tag="pt", name=f"pt{vb}_{h}")
            nc.tensor.matmul(
                out=pt,
                lhsT=u_sb[:, vb * R + h * 128 : vb * R + h * 128 + 128],
                rhs=v_sb[:, vb * L2 : (vb + 1) * L2],
                start=True,
                stop=True,
            )
            dst = stage[vb % 4]
            if eng == "act":
                nc.scalar.copy(out=dst, in_=pt)
            else:
                nc.vector.tensor_copy(out=dst, in_=pt)
            # stage -> dram (partition-major on the SBUF side)
            nc.sync.dma_start(
                out=c_rt[vb, 128 * h : 128 * h + 128, :], in_=dst
            )

        def emit_group_dma(g, h):
            # dram -> csk: iteration (vb, row, col) on both sides
            for q in range(NQ):
                in_ap = c_rt[
                    g * GRP : (g + 1) * GRP,
                    128 * h : 128 * h + 128,
                    q * W : (q + 1) * W,
                ]
                base = (128 * h + 2 * q) * W
                out_ap = csk[
                    q * 32 + g * GRP : q * 32 + g * GRP + GRP,
                    base : base + 128 * W,
                ].rearrange("v (r j) -> v r j", r=128)
                nc.sync.dma_start(out=out_ap, in_=in_ap)

        # --- h=0 production (pipeline fill) ---
        for vb in range(VB):
            emit_mm_conv(vb, 0, "dve" if vb % 2 else "act")
            if vb % GRP == GRP - 1:
                emit_group_dma(vb // GRP, 0)

        # --- DP loop with h=1 production interleaved ---
        for s in range(NSS):
            if s >= 4 and (s - 4) % 3 == 0:
                i = (s - 4) // 3
                if i < VB:
                    emit_mm_conv(i, 1, "act")
                    if i % GRP == GRP - 1:
                        emit_group_dma(i // GRP, 1)
            if s == 0:
                d0 = bigm
                ini = init0[:, 0:1]
            else:
                pb = new[(s - 1) % 4]
                mb = mm[s % 2]
                nc.vector.scalar_tensor_tensor(
                    out=mb, in0=pb[:, 1 : W + 1], scalar=0.0,
                    in1=pb[:, 0:W], op0=Alu.bypass, op1=Alu.min,
                )
                if s in (2, 4, 6):
                    q = s // 2
                    nc.vector.memset(mb[q * 32 : (q + 1) * 32, :], BIG)
                d0 = mb
                ini = new[s % 4][:, 0:1]
            nc.vector.tensor_tensor_scan(
                out=new[s % 4][:, 1 : W + 1],
                data0=d0,
                data1=csk[:, s * W : (s + 1) * W],
                initial=ini,
                op0=Alu.min,
                op1=Alu.add,
            )
            if s + 2 < NSS:
                tgt = new[(s + 2) % 4]
                nb = new[s % 4]
                nc.scalar.copy(out=tgt[32:64, 0:1], in_=nb[0:32, W : W + 1])
                nc.gpsimd.tensor_copy(out=tgt[64:96, 0:1], in_=nb[32:64, W : W + 1])
                nc.gpsimd.tensor_copy(out=tgt[96:128, 0:1], in_=nb[64:96, W : W + 1])
            if s >= R - 1 and (s - (R - 1)) % 2 == 0 and (s - (R - 1)) // 2 < NQ:
                q = (s - (R - 1)) // 2
                nc.scalar.copy(
                    out=outbuf[0:32, q * W : (q + 1) * W],
                    in_=new[s % 4][q * 32 : (q + 1) * 32, 1 : W + 1],
                )
        nc.sync.dma_start(out=out_d, in_=outbuf)


def _build():
    import concourse.bacc as bacc
    import concourse.tile as tile
    from concourse import mybir

    F32 = mybir.dt.float32
    BF16 = mybir.dt.bfloat16
    nc = bacc.Bacc()
    u_d = nc.dram_tensor("u_all", [KAUG, VB * R], BF16, kind="ExternalInput")[:]
    v_d = nc.dram_tensor("v_all", [KAUG, VB * L2], BF16, kind="ExternalInput")[:]
    out_d = nc.dram_tensor("out_rows", [VB, L2], F32, kind="ExternalOutput")[:]
    with tile.TileContext(nc) as tc:
        _emit(tc, u_d, v_d, out_d)
    nc.compile()
    return nc


def _host_prep(s1, s2):
    """Per-core u_all [18, 32*256] and v_all [18, 32*512] bf16 tensors."""
    import ml_dtypes

    BF = ml_dtypes.bfloat16
    s1 = np.ascontiguousarray(s1, dtype=np.float32)
    s2 = np.ascontiguousarray(s2, dtype=np.float32)
    in_maps = []
    for c in range(N_CORES):
        s1c = s1[c * PER_CORE : (c + 1) * PER_CORE]  # [16, 512, 16]
        s2c = s2[c * PER_CORE : (c + 1) * PER_CORE]
        s1v = np.concatenate([s1c[:, :R], s1c[:, ::-1][:, :R]], axis=0)  # [32,256,16]
        s2v = np.concatenate([s2c, s2c[:, ::-1]], axis=0)  # [32,512,16]
        u = np.empty((VB, R, KAUG), np.float32)
        u[:, :, :D] = -2.0 * s1v
        u[:, :, D] = 1.0
        u[:, :, D + 1] = (s1v * s1v).sum(-1)
        v = np.empty((VB, L2, KAUG), np.float32)
        v[:, :, :D] = s2v
        v[:, :, D] = (s2v * s2v).sum(-1)
        v[:, :, D + 1] = 1.0
        in_maps.append(
            {
                "u_all": np.ascontiguousarray(u.transpose(2, 0, 1).reshape(KAUG, VB * R)).astype(BF),
                "v_all": np.ascontiguousarray(v.transpose(2, 0, 1).reshape(KAUG, VB * L2)).astype(BF),
            }
        )
    return in_maps


def _combine(outs):
    """outs: list of [VB, 512] final-row arrays per core -> scalar loss."""
    vals = np.empty(B, np.float64)
    for c in range(N_CORES):
        rows = outs[c]
        for bl in range(PER_CORE):
            F = rows[bl].astype(np.float64)
            Brow = rows[PER_CORE + bl][::-1].astype(np.float64)
            Bnext = np.concatenate([Brow[1:], [np.inf]])
            vals[c * PER_CORE + bl] = np.min(F + np.minimum(Brow, Bnext))
    return np.float32(np.mean(np.sqrt(vals)))


def kernel(s1_batch, s2_batch):
    from concourse import bass_utils

    if "nc" not in _CACHE:
        _CACHE["nc"] = _build()
    nc = _CACHE["nc"]
    in_maps = _host_prep(np.asarray(s1_batch), np.asarray(s2_batch))
    kw = {}
    if _CACHE.get("trace"):
        kw = dict(trace=True, trace_cores=_CACHE.get("trace_cores", [0]),
                  tmpdir=_CACHE.get("tmpdir"))
    res = bass_utils.run_bass_kernel_spmd(
        nc, in_maps, core_ids=list(range(N_CORES)), **kw
    )
    if res.exec_time_ns is not None:
        _CACHE["exec_time_ns"] = res.exec_time_ns
    _CACHE["last_results"] = res
    outs = [r["out_rows"] for r in res.results]
    return _combine(outs)


# revision 10
# speedup vs baseline: 1.0993x; 1.0993x over previous
"""DTW loss kernel for Trainium2 (8 NeuronCores, Bass/Tile) — v2.

reference: C[b,i,j] = ||s1[b,i]-s2[b,j]||^2 ; DTW DP over [512,512]; return
mean_b sqrt(DTW[b,-1,-1]).

Meet-in-the-middle: DTW_end = min_j F[255,j] + min(B[256,j], B[256,j+1]) where
F is the forward DP over rows 0..255 and B the backward DP. Each core: 16
batch elems x 2 directions = 32 virtual batches (vb) of 256-row half-DPs.

v2 design (vs v1's per-superstep psum gather copies):
- Cost rows come from per-vb matmuls: lhsT = u[vb] chunk [18,128],
  rhs = v[vb] [18,512] -> psum [128 rows, 512 cols] f32. 64 matmuls total
  (5x fewer PE cycles than v1's block-diagonal batching, whose cost was
  dominated by 5 K-chunk passes over the 512-wide output).
- Act/DVE convert psum -> fp16 staging [rows, 512].
- Skewed cost buffer csk[q*32+vb, s*W + j] = C[vb, s-2q, q*W+j] in SBUF.
  A single SBUF->SBUF DMA cannot move data across the partition-dim meaning
  (SBUF APs are partition-major), so the reshape takes a DRAM round-trip:
  stage -> dram c_rt[vb, r, j] per vb, then dram -> csk per (band, vb-group)
  with matching flat iteration orders. The DP scan then reads data1 directly
  from csk with zero per-superstep copies (v1 burned the Scalar engine on 3
  gather copies per superstep).
- Wavefront: NQ=4 bands of W=128 cols on 128 partitions = (band q, vb).
  Superstep s: band q scans row s-2q. Carries between bands ride in col 0
  of the `new` tiles (one [32,1] partition-shifted copy per band boundary,
  2 supersteps of slack, on Act/Pool engines).
"""

import numpy as np

B = 128
L1 = 512
L2 = 512
D = 16
N_CORES = 8
PER_CORE = B // N_CORES  # 16
VB = 2 * PER_CORE  # 32
R = L1 // 2  # 256
KAUG = D + 2  # 18
NQ = 4
W = L2 // NQ  # 128
NSS = R + 2 * (NQ - 1)  # 262
GRP = 8  # vb per staging group
NGRP = VB // GRP  # 4
BIG = 1e30

_CACHE = {}


def _emit(tc, u_d, v_d, out_d):
    import concourse.bass as bass  # noqa: F401
    from concourse import mybir

    F32 = mybir.dt.float32
    F16 = mybir.dt.float16
    BF16 = mybir.dt.bfloat16
    Alu = mybir.AluOpType
    nc = tc.nc

    with (
        tc.tile_pool(name="singles", bufs=1) as sp,
        tc.tile_pool(name="psum", bufs=4, space="PSUM") as pp,
        tc.tile_pool(name="dramp", bufs=1, space="DRAM") as dp,
    ):
        c_rt = dp.tile([VB, R, L2], mybir.dt.float16, tag="crt", name="crt")
        u_sb = sp.tile([KAUG, VB * R], BF16, tag="u", name="u")
        v_sb = sp.tile([KAUG, VB * L2], BF16, tag="v", name="v")
        csk = sp.tile([128, NSS * W], F16, tag="csk", name="csk")
        stage = [
            sp.tile([128, L2], F16, tag=f"stg{i}", name=f"stg{i}")
            for i in range(4)
        ]
        new = [sp.tile([128, W + 1], F32, tag=f"new{p}", name=f"new{p}") for p in range(4)]
        mm = [sp.tile([128, W], F32, tag=f"m{p}", name=f"m{p}") for p in range(2)]
        bigm = sp.tile([128, W], F32, tag="bigm", name="bigm")
        init0 = sp.tile([128, 1], F32, tag="init0", name="init0")
        outbuf = sp.tile([32, L2], F32, tag="ob", name="ob")

        # --- prologue ---
        nc.sync.dma_start(out=u_sb, in_=u_d)
        nc.sync.dma_start(out=v_sb, in_=v_d)
        nc.vector.memset(bigm, BIG)
        nc.vector.memset(init0, 0.0)
        for p in range(4):
            nc.vector.memset(new[p][:, 0:1], BIG)
        # csk margins (supersteps where a band has no valid row)
        nc.vector.memset(csk[:, 0 : 2 * (NQ - 1) * W], 0.0)
        nc.vector.memset(csk[:, R * W : NSS * W], 0.0)

        def emit_mm_conv(vb, h, eng):
            pt = pp.tile([128, L2], F32, tag="pt", name=f"pt{vb}_{h}")
            nc.tensor.matmul(
                out=pt,
                lhsT=u_sb[:, vb * R + h * 128 : vb * R + h * 128 + 128],
                rhs=v_sb[:, vb * L2 : (vb + 1) * L2],
                start=True,
                stop=True,
            )
            dst = stage[vb % 4]
            if eng == "act":
                nc.scalar.copy(out=dst, in_=pt)
            else:
                nc.vector.tensor_copy(out=dst, in_=pt)
            # stage -> dram (partition-major on the SBUF side)
            nc.sync.dma_start(
                out=c_rt[vb, 128 * h : 128 * h + 128, :], in_=dst
            )

        def emit_group_dma(g, h, q):
            # dram -> csk: iteration (vb, row, col) on both sides. Triggered
            # via gpsimd SWDGE: 0.34ns/descriptor vs HWDGE's ~6ns (these are
            # 256B-descriptor transfers, 1024 descriptors each).
            in_ap = c_rt[
                g * GRP : (g + 1) * GRP,
                128 * h : 128 * h + 128,
                q * W : (q + 1) * W,
            ]
            base = (128 * h + 2 * q) * W
            out_ap = csk[
                q * 32 + g * GRP : q * 32 + g * GRP + GRP,
                base : base + 128 * W,
            ].rearrange("v (r j) -> v r j", r=128)
            nc.gpsimd.dma_start(out=out_ap, in_=in_ap)

        # --- h=0 production (pipeline fill) ---
        for vb in range(VB):
            emit_mm_conv(vb, 0, "dve" if vb % 2 else "act")
            if vb % GRP == GRP - 1:
                for q in range(NQ):
                    emit_group_dma(vb // GRP, 0, q)

        # --- DP loop with h=1 production interleaved ---
        # pacing: one matmul+convert every 3 supersteps; the 4 reshape DMAs
        # of a finished group spread one per superstep to avoid pool bursts
        dma_q = []  # pending (g, h, q) triggers
        for s in range(NSS):
            if s >= 4 and (s - 4) % 3 == 0:
                i = (s - 4) // 3
                if i < VB:
                    emit_mm_conv(i, 1, "act")
                    if i % GRP == GRP - 1:
                        dma_q.extend((i // GRP, 1, q) for q in range(NQ))
            if dma_q:
                emit_group_dma(*dma_q.pop(0))
            if s == 0:
                d0 = bigm
                ini = init0[:, 0:1]
            else:
                pb = new[(s - 1) % 4]
                mb = mm[s % 2]
                nc.vector.scalar_tensor_tensor(
                    out=mb, in0=pb[:, 1 : W + 1], scalar=0.0,
                    in1=pb[:, 0:W], op0=Alu.bypass, op1=Alu.min,
                )
                if s in (2, 4, 6):
                    q = s // 2
                    nc.vector.memset(mb[q * 32 : (q + 1) * 32, :], BIG)
                d0 = mb
                ini = new[s % 4][:, 0:1]
            nc.vector.tensor_tensor_scan(
                out=new[s % 4][:, 1 : W + 1],
                data0=d0,
                data1=csk[:, s * W : (s + 1) * W],
                initial=ini,
                op0=Alu.min,
                op1=Alu.add,
            )
            if s + 2 < NSS:
                tgt = new[(s + 2) % 4]
                nb = new[s % 4]
                nc.scalar.copy(out=tgt[32:64, 0:1], in_=nb[0:32, W : W + 1])
                nc.gpsimd.tensor_copy(out=tgt[64:96, 0:1], in_=nb[32:64, W : W + 1])
                nc.gpsimd.tensor_copy(out=tgt[96:128, 0:1], in_=nb[64:96, W : W + 1])
            if s >= R - 1 and (s - (R - 1)) % 2 == 0 and (s - (R - 1)) // 2 < NQ:
                q = (s - (R - 1)) // 2
                nc.scalar.copy(
                    out=outbuf[0:32, q * W : (q + 1) * W],
                    in_=new[s % 4][q * 32 : (q + 1) * 32, 1 : W + 1],
                )
        nc.sync.dma_start(out=out_d, in_=outbuf)


def _build():
    import concourse.bacc as bacc
    import concourse.tile as tile
    from concourse import mybir

    F32 = mybir.dt.float32
    BF16 = mybir.dt.bfloat16
    nc = bacc.Bacc()
    u_d = nc.dram_tensor("u_all", [KAUG, VB * R], BF16, kind="ExternalInput")[:]
    v_d = nc.dram_tensor("v_all", [KAUG, VB * L2], BF16, kind="ExternalInput")[:]
    out_d = nc.dram_tensor("out_rows", [VB, L2], F32, kind="ExternalOutput")[:]
    with tile.TileContext(nc) as tc:
        _emit(tc, u_d, v_d, out_d)
    nc.compile()
    return nc


def _host_prep(s1, s2):
    """Per-core u_all [18, 32*256] and v_all [18, 32*512] bf16 tensors."""
    import ml_dtypes

    BF = ml_dtypes.bfloat16
    s1 = np.ascontiguousarray(s1, dtype=np.float32)
    s2 = np.ascontiguousarray(s2, dtype=np.float32)
    in_maps = []
    for c in range(N_CORES):
        s1c = s1[c * PER_CORE : (c + 1) * PER_CORE]  # [16, 512, 16]
        s2c = s2[c * PER_CORE : (c + 1) * PER_CORE]
        s1v = np.concatenate([s1c[:, :R], s1c[:, ::-1][:, :R]], axis=0)  # [32,256,16]
        s2v = np.concatenate([s2c, s2c[:, ::-1]], axis=0)  # [32,512,16]
        u = np.empty((VB, R, KAUG), np.float32)
        u[:, :, :D] = -2.0 * s1v
        u[:, :, D] = 1.0
        u[:, :, D + 1] = (s1v * s1v).sum(-1)
        v = np.empty((VB, L2, KAUG), np.float32)
        v[:, :, :D] = s2v
        v[:, :, D] = (s2v * s2v).sum(-1)
        v[:, :, D + 1] = 1.0
        in_maps.append(
            {
                "u_all": np.ascontiguousarray(u.transpose(2, 0, 1).reshape(KAUG, VB * R)).astype(BF),
                "v_all": np.ascontiguousarray(v.transpose(2, 0, 1).reshape(KAUG, VB * L2)).astype(BF),
            }
        )
    return in_maps


def _combine(outs):
    """outs: list of [VB, 512] final-row arrays per core -> scalar loss."""
    vals = np.empty(B, np.float64)
    for c in range(N_CORES):
        rows = outs[c]
        for bl in range(PER_CORE):
            F = rows[bl].astype(np.float64)
            Brow = rows[PER_CORE + bl][::-1].astype(np.float64)
            Bnext = np.concatenate([Brow[1:], [np.inf]])
            vals[c * PER_CORE + bl] = np.min(F + np.minimum(Brow, Bnext))
    return np.float32(np.mean(np.sqrt(vals)))


def kernel(s1_batch, s2_batch):
    from concourse import bass_utils

    if "nc" not in _CACHE:
        _CACHE["nc"] = _build()
    nc = _CACHE["nc"]
    in_maps = _host_prep(np.asarray(s1_batch), np.asarray(s2_batch))
    kw = {}
    if _CACHE.get("trace"):
        kw = dict(trace=True, trace_cores=_CACHE.get("trace_cores", [0]),
                  tmpdir=_CACHE.get("tmpdir"))
    res = bass_utils.run_bass_kernel_spmd(
        nc, in_maps, core_ids=list(range(N_CORES)), **kw
    )
    if res.exec_time_ns is not None:
        _CACHE["exec_time_ns"] = res.exec_time_ns
    _CACHE["last_results"] = res
    outs = [r["out_rows"] for r in res.results]
    return _combine(outs)


# revision 14
# speedup vs baseline: 1.3073x; 1.1892x over previous
"""DTW loss kernel for Trainium2 (8 NeuronCores, Bass/Tile) — v2.

reference: C[b,i,j] = ||s1[b,i]-s2[b,j]||^2 ; DTW DP over [512,512]; return
mean_b sqrt(DTW[b,-1,-1]).

Meet-in-the-middle: DTW_end = min_j F[255,j] + min(B[256,j], B[256,j+1]) where
F is the forward DP over rows 0..255 and B the backward DP. Each core: 16
batch elems x 2 directions = 32 virtual batches (vb) of 256-row half-DPs.

v2 design (vs v1's per-superstep psum gather copies):
- Cost rows come from per-vb matmuls: lhsT = u[vb] chunk [18,128],
  rhs = v[vb] [18,512] -> psum [128 rows, 512 cols] f32. 64 matmuls total
  (5x fewer PE cycles than v1's block-diagonal batching, whose cost was
  dominated by 5 K-chunk passes over the 512-wide output).
- Act/DVE convert psum -> fp16 staging [rows, 512].
- Skewed cost buffer csk[q*32+vb, s*W + j] = C[vb, s-2q, q*W+j] in SBUF.
  A single SBUF->SBUF DMA cannot move data across the partition-dim meaning
  (SBUF APs are partition-major), so the reshape takes a DRAM round-trip:
  stage -> dram c_rt[vb, r, j] per vb, then dram -> csk per (band, vb-group)
  with matching flat iteration orders. The DP scan then reads data1 directly
  from csk with zero per-superstep copies (v1 burned the Scalar engine on 3
  gather copies per superstep).
- Wavefront: NQ=4 bands of W=128 cols on 128 partitions = (band q, vb).
  Superstep s: band q scans row s-2q. Carries between bands ride in col 0
  of the `new` tiles (one [32,1] partition-shifted copy per band boundary,
  2 supersteps of slack, on Act/Pool engines).
"""

import numpy as np

B = 128
L1 = 512
L2 = 512
D = 16
N_CORES = 8
PER_CORE = B // N_CORES  # 16
VB = 2 * PER_CORE  # 32
R = L1 // 2  # 256
KAUG = D + 2  # 18
NQ = 4
W = L2 // NQ  # 128
NSS = R + 2 * (NQ - 1)  # 262
GRP = 8  # vb per staging group
NGRP = VB // GRP  # 4
BIG = 1e30

_CACHE = {}


def _emit(tc, u_d, v_d, out_d):
    import concourse.bass as bass  # noqa: F401
    from concourse import mybir

    F32 = mybir.dt.float32
    F16 = mybir.dt.float16
    BF16 = mybir.dt.bfloat16
    Alu = mybir.AluOpType
    nc = tc.nc

    with (
        tc.tile_pool(name="singles", bufs=1) as sp,
        tc.tile_pool(name="psum", bufs=4, space="PSUM") as pp,
        tc.tile_pool(name="dramp", bufs=1, space="DRAM") as dp,
    ):
        # band-major DRAM scratch: c_rt2[vb, q, r, j] = C[vb, r, q*W+j].
        # DMA1 (stage->dram) pays the 256B-strided side as posted writes;
        # DMA2 (dram->csk) then reads contiguous 32KB runs per (vb,q).
        c_rt = dp.tile([VB, NQ, R, W], mybir.dt.float16, tag="crt", name="crt")
        u_sb = sp.tile([KAUG, VB * R], BF16, tag="u", name="u")
        v_sb = sp.tile([KAUG, VB * L2], BF16, tag="v", name="v")
        csk = sp.tile([128, NSS * W], F16, tag="csk", name="csk")
        stage = [
            sp.tile([128, GRP * L2], F16, tag=f"stg{i}", name=f"stg{i}")
            for i in range(2)
        ]
        new = [sp.tile([128, W + 1], F32, tag=f"new{p}", name=f"new{p}") for p in range(4)]
        mm = [sp.tile([128, W], F32, tag=f"m{p}", name=f"m{p}") for p in range(2)]
        bigm = sp.tile([128, W], F32, tag="bigm", name="bigm")
        init0 = sp.tile([128, 1], F32, tag="init0", name="init0")
        outbuf = sp.tile([32, L2], F32, tag="ob", name="ob")

        # --- prologue ---
        nc.sync.dma_start(out=u_sb, in_=u_d)
        nc.sync.dma_start(out=v_sb, in_=v_d)
        nc.vector.memset(bigm, BIG)
        nc.vector.memset(init0, 0.0)
        for p in range(4):
            nc.vector.memset(new[p][:, 0:1], BIG)
        # csk margins (supersteps where a band has no valid row)
        nc.vector.memset(csk[:, 0 : 2 * (NQ - 1) * W], 0.0)
        nc.vector.memset(csk[:, R * W : NSS * W], 0.0)

        def emit_mm_conv(vb, h, eng):
            pt = pp.tile([128, L2], F32, tag="pt", name=f"pt{vb}_{h}")
            nc.tensor.matmul(
                out=pt,
                lhsT=u_sb[:, vb * R + h * 128 : vb * R + h * 128 + 128],
                rhs=v_sb[:, vb * L2 : (vb + 1) * L2],
                start=True,
                stop=True,
            )
            g = vb // GRP
            dst = stage[g % 2][:, (vb % GRP) * L2 : (vb % GRP + 1) * L2]
            if eng == "act":
                nc.scalar.copy(out=dst, in_=pt)
            else:
                nc.vector.tensor_copy(out=dst, in_=pt)

        def emit_stage_dma(g, h):
            # stage [r-part, (vb,q,j)] -> c_rt2[vb, q, 128h+r, j]: iteration
            # (r, (vb,q), j); out (vb,q) merges to one uniform-stride dim.
            # SWDGE (gpsimd) so the 4096 256B descriptors cost 0.34ns each.
            buf = stage[g % 2]
            out_ap = c_rt[g * GRP : (g + 1) * GRP, :, 128 * h : 128 * h + 128, :]
            out_ap = out_ap.rearrange("v q r j -> r (v q) j")
            in_ap = buf[:, :].rearrange("r (vq j) -> r vq j", j=W)
            nc.gpsimd.dma_start(out=out_ap, in_=in_ap)

        def emit_group_dma(g, h, q):
            # dram -> csk: contiguous 32KB runs per vb on the dram side
            in_ap = c_rt[g * GRP : (g + 1) * GRP, q, 128 * h : 128 * h + 128, :]
            base = (128 * h + 2 * q) * W
            out_ap = csk[
                q * 32 + g * GRP : q * 32 + g * GRP + GRP,
                base : base + 128 * W,
            ].rearrange("v (r j) -> v r j", r=128)
            nc.sync.dma_start(out=out_ap, in_=in_ap)

        # --- h=0 production (pipeline fill) ---
        for vb in range(VB):
            emit_mm_conv(vb, 0, "dve" if vb % 2 else "act")
            if vb % GRP == GRP - 1:
                emit_stage_dma(vb // GRP, 0)
                for q in range(NQ):
                    emit_group_dma(vb // GRP, 0, q)

        # --- DP loop with h=1 production interleaved ---
        # pacing: one matmul+convert every 3 supersteps; a finished group's
        # stage->dram DMA then its 4 dram->csk DMAs, one per superstep
        dma_q = []  # pending emitters
        for s in range(NSS):
            if s >= 4 and (s - 4) % 3 == 0:
                i = (s - 4) // 3
                if i < VB:
                    emit_mm_conv(i, 1, "act")
                    if i % GRP == GRP - 1:
                        g = i // GRP
                        dma_q.append((emit_stage_dma, (g, 1)))
                        dma_q.extend(
                            (emit_group_dma, (g, 1, q)) for q in range(NQ)
                        )
            if dma_q:
                fn, args = dma_q.pop(0)
                fn(*args)
            if s == 0:
                d0 = bigm
                ini = init0[:, 0:1]
            else:
                pb = new[(s - 1) % 4]
                mb = mm[s % 2]
                nc.vector.scalar_tensor_tensor(
                    out=mb, in0=pb[:, 1 : W + 1], scalar=0.0,
                    in1=pb[:, 0:W], op0=Alu.bypass, op1=Alu.min,
                )
                if s in (2, 4, 6):
                    q = s // 2
                    nc.vector.memset(mb[q * 32 : (q + 1) * 32, :], BIG)
                d0 = mb
                ini = new[s % 4][:, 0:1]
            nc.vector.tensor_tensor_scan(
                out=new[s % 4][:, 1 : W + 1],
                data0=d0,
                data1=csk[:, s * W : (s + 1) * W],
                initial=ini,
                op0=Alu.min,
                op1=Alu.add,
            )
            if s + 2 < NSS:
                tgt = new[(s + 2) % 4]
                nb = new[s % 4]
                nc.scalar.copy(out=tgt[32:64, 0:1], in_=nb[0:32, W : W + 1])
                nc.gpsimd.tensor_copy(out=tgt[64:96, 0:1], in_=nb[32:64, W : W + 1])
                nc.gpsimd.tensor_copy(out=tgt[96:128, 0:1], in_=nb[64:96, W : W + 1])
            if s >= R - 1 and (s - (R - 1)) % 2 == 0 and (s - (R - 1)) // 2 < NQ:
                q = (s - (R - 1)) // 2
                nc.scalar.copy(
                    out=outbuf[0:32, q * W : (q + 1) * W],
                    in_=new[s % 4][q * 32 : (q + 1) * 32, 1 : W + 1],
                )
        nc.sync.dma_start(out=out_d, in_=outbuf)


def _build():
    import concourse.bacc as bacc
    import concourse.tile as tile
    from concourse import mybir

    F32 = mybir.dt.float32
    BF16 = mybir.dt.bfloat16
    nc = bacc.Bacc()
    u_d = nc.dram_tensor("u_all", [KAUG, VB * R], BF16, kind="ExternalInput")[:]
    v_d = nc.dram_tensor("v_all", [KAUG, VB * L2], BF16, kind="ExternalInput")[:]
    out_d = nc.dram_tensor("out_rows", [VB, L2], F32, kind="ExternalOutput")[:]
    with tile.TileContext(nc) as tc:
        _emit(tc, u_d, v_d, out_d)
    nc.compile()
    return nc


def _host_prep(s1, s2):
    """Per-core u_all [18, 32*256] and v_all [18, 32*512] bf16 tensors."""
    import ml_dtypes

    BF = ml_dtypes.bfloat16
    s1 = np.ascontiguousarray(s1, dtype=np.float32)
    s2 = np.ascontiguousarray(s2, dtype=np.float32)
    in_maps = []
    for c in range(N_CORES):
        s1c = s1[c * PER_CORE : (c + 1) * PER_CORE]  # [16, 512, 16]
        s2c = s2[c * PER_CORE : (c + 1) * PER_CORE]
        s1v = np.concatenate([s1c[:, :R], s1c[:, ::-1][:, :R]], axis=0)  # [32,256,16]
        s2v = np.concatenate([s2c, s2c[:, ::-1]], axis=0)  # [32,512,16]
        u = np.empty((VB, R, KAUG), np.float32)
        u[:, :, :D] = -2.0 * s1v
        u[:, :, D] = 1.0
        u[:, :, D + 1] = (s1v * s1v).sum(-1)
        v = np.empty((VB, L2, KAUG), np.float32)
        v[:, :, :D] = s2v
        v[:, :, D] = (s2v * s2v).sum(-1)
        v[:, :, D + 1] = 1.0
        in_maps.append(
            {
                "u_all": np.ascontiguousarray(u.transpose(2, 0, 1).reshape(KAUG, VB * R)).astype(BF),
                "v_all": np.ascontiguousarray(v.transpose(2, 0, 1).reshape(KAUG, VB * L2)).astype(BF),
            }
        )
    return in_maps


def _combine(outs):
    """outs: list of [VB, 512] final-row arrays per core -> scalar loss."""
    vals = np.empty(B, np.float64)
    for c in range(N_CORES):
        rows = outs[c]
        for bl in range(PER_CORE):
            F = rows[bl].astype(np.float64)
            Brow = rows[PER_CORE + bl][::-1].astype(np.float64)
            Bnext = np.concatenate([Brow[1:], [np.inf]])
            vals[c * PER_CORE + bl] = np.min(F + np.minimum(Brow, Bnext))
    return np.float32(np.mean(np.sqrt(vals)))


def kernel(s1_batch, s2_batch):
    from concourse import bass_utils

    if "nc" not in _CACHE:
        _CACHE["nc"] = _build()
    nc = _CACHE["nc"]
    in_maps = _host_prep(np.asarray(s1_batch), np.asarray(s2_batch))
    kw = {}
    if _CACHE.get("trace"):
        kw = dict(trace=True, trace_cores=_CACHE.get("trace_cores", [0]),
                  tmpdir=_CACHE.get("tmpdir"))
    res = bass_utils.run_bass_kernel_spmd(
        nc, in_maps, core_ids=list(range(N_CORES)), **kw
    )
    if res.exec_time_ns is not None:
        _CACHE["exec_time_ns"] = res.exec_time_ns
    _CACHE["last_results"] = res
    outs = [r["out_rows"] for r in res.results]
    return _combine(outs)


# revision 16
# speedup vs baseline: 1.3637x; 1.0432x over previous
"""DTW loss kernel for Trainium2 (8 NeuronCores, Bass/Tile) — v2.

reference: C[b,i,j] = ||s1[b,i]-s2[b,j]||^2 ; DTW DP over [512,512]; return
mean_b sqrt(DTW[b,-1,-1]).

Meet-in-the-middle: DTW_end = min_j F[255,j] + min(B[256,j], B[256,j+1]) where
F is the forward DP over rows 0..255 and B the backward DP. Each core: 16
batch elems x 2 directions = 32 virtual batches (vb) of 256-row half-DPs.

v2 design (vs v1's per-superstep psum gather copies):
- Cost rows come from per-vb matmuls: lhsT = u[vb] chunk [18,128],
  rhs = v[vb] [18,512] -> psum [128 rows, 512 cols] f32. 64 matmuls total
  (5x fewer PE cycles than v1's block-diagonal batching, whose cost was
  dominated by 5 K-chunk passes over the 512-wide output).
- Act/DVE convert psum -> fp16 staging [rows, 512].
- Skewed cost buffer csk[q*32+vb, s*W + j] = C[vb, s-2q, q*W+j] in SBUF.
  A single SBUF->SBUF DMA cannot move data across the partition-dim meaning
  (SBUF APs are partition-major), so the reshape takes a DRAM round-trip:
  stage -> dram c_rt[vb, r, j] per vb, then dram -> csk per (band, vb-group)
  with matching flat iteration orders. The DP scan then reads data1 directly
  from csk with zero per-superstep copies (v1 burned the Scalar engine on 3
  gather copies per superstep).
- Wavefront: NQ=4 bands of W=128 cols on 128 partitions = (band q, vb).
  Superstep s: band q scans row s-2q. Carries between bands ride in col 0
  of the `new` tiles (one [32,1] partition-shifted copy per band boundary,
  2 supersteps of slack, on Act/Pool engines).
"""

import numpy as np

B = 128
L1 = 512
L2 = 512
D = 16
N_CORES = 8
PER_CORE = B // N_CORES  # 16
VB = 2 * PER_CORE  # 32
R = L1 // 2  # 256
KAUG = D + 2  # 18
NQ = 4
W = L2 // NQ  # 128
NSS = R + 2 * (NQ - 1)  # 262
GRP = 8  # vb per staging group
NGRP = VB // GRP  # 4
BIG = 1e30

_CACHE = {}


def _emit(tc, u_d, v_d, out_d):
    import concourse.bass as bass  # noqa: F401
    from concourse import mybir

    F32 = mybir.dt.float32
    F16 = mybir.dt.float16
    BF16 = mybir.dt.bfloat16
    Alu = mybir.AluOpType
    nc = tc.nc

    with (
        tc.tile_pool(name="singles", bufs=1) as sp,
        tc.tile_pool(name="psum", bufs=4, space="PSUM") as pp,
        tc.tile_pool(name="dramp", bufs=1, space="DRAM") as dp,
    ):
        # band-major DRAM scratch: c_rt2[vb, q, r, j] = C[vb, r, q*W+j].
        # DMA1 (stage->dram) pays the 256B-strided side as posted writes;
        # DMA2 (dram->csk) then reads contiguous 32KB runs per (vb,q).
        c_rt = dp.tile([VB, NQ, R, W], mybir.dt.float16, tag="crt", name="crt")
        u_sb = sp.tile([KAUG, VB * R], BF16, tag="u", name="u")
        v_sb = sp.tile([KAUG, VB * L2], BF16, tag="v", name="v")
        csk = sp.tile([128, NSS * W], F16, tag="csk", name="csk")
        stage = [
            sp.tile([128, GRP * L2], F16, tag=f"stg{i}", name=f"stg{i}")
            for i in range(4)
        ]
        new = [sp.tile([128, W + 1], F32, tag=f"new{p}", name=f"new{p}") for p in range(4)]
        mm = [sp.tile([128, W], F32, tag=f"m{p}", name=f"m{p}") for p in range(2)]
        bigm = sp.tile([128, W], F32, tag="bigm", name="bigm")
        init0 = sp.tile([128, 1], F32, tag="init0", name="init0")
        outbuf = sp.tile([32, L2], F32, tag="ob", name="ob")

        # --- prologue ---
        nc.sync.dma_start(out=u_sb, in_=u_d)
        nc.sync.dma_start(out=v_sb, in_=v_d)
        nc.vector.memset(bigm, BIG)
        nc.vector.memset(init0, 0.0)
        for p in range(4):
            nc.vector.memset(new[p][:, 0:1], BIG)
        # csk margins (supersteps where a band has no valid row)
        nc.vector.memset(csk[:, 0 : 2 * (NQ - 1) * W], 0.0)
        nc.vector.memset(csk[:, R * W : NSS * W], 0.0)

        def emit_mm_conv(vb, h, eng):
            pt = pp.tile([128, L2], F32, tag="pt", name=f"pt{vb}_{h}")
            nc.tensor.matmul(
                out=pt,
                lhsT=u_sb[:, vb * R + h * 128 : vb * R + h * 128 + 128],
                rhs=v_sb[:, vb * L2 : (vb + 1) * L2],
                start=True,
                stop=True,
            )
            g = vb // GRP
            dst = stage[g % 4][:, (vb % GRP) * L2 : (vb % GRP + 1) * L2]
            if eng == "act":
                nc.scalar.copy(out=dst, in_=pt)
            else:
                nc.vector.tensor_copy(out=dst, in_=pt)

        def emit_stage_dma(g, h):
            # stage [r-part, (vb,q,j)] -> c_rt2[vb, q, 128h+r, j]: iteration
            # (r, (vb,q), j); out (vb,q) merges to one uniform-stride dim.
            # SWDGE (gpsimd) so the 4096 256B descriptors cost 0.34ns each.
            buf = stage[g % 4]
            out_ap = c_rt[g * GRP : (g + 1) * GRP, :, 128 * h : 128 * h + 128, :]
            out_ap = out_ap.rearrange("v q r j -> r (v q) j")
            in_ap = buf[:, :].rearrange("r (vq j) -> r vq j", j=W)
            nc.gpsimd.dma_start(out=out_ap, in_=in_ap)

        def emit_group_dma(g, h, q):
            # dram -> csk: contiguous 32KB runs per vb on the dram side
            in_ap = c_rt[g * GRP : (g + 1) * GRP, q, 128 * h : 128 * h + 128, :]
            base = (128 * h + 2 * q) * W
            out_ap = csk[
                q * 32 + g * GRP : q * 32 + g * GRP + GRP,
                base : base + 128 * W,
            ].rearrange("v (r j) -> v r j", r=128)
            nc.sync.dma_start(out=out_ap, in_=in_ap)

        # --- h=0 production (pipeline fill) ---
        for vb in range(VB):
            emit_mm_conv(vb, 0, "dve" if vb % 2 else "act")
            if vb % GRP == GRP - 1:
                emit_stage_dma(vb // GRP, 0)
                for q in range(NQ):
                    emit_group_dma(vb // GRP, 0, q)

        # --- DP loop with h=1 production interleaved ---
        # pacing: one matmul+convert every 3 supersteps; a finished group's
        # stage->dram DMA then its 4 dram->csk DMAs, one per superstep
        dma_q = []  # pending emitters
        for s in range(NSS):
            if s >= 4 and (s - 4) % 3 == 0:
                i = (s - 4) // 3
                if i < VB:
                    emit_mm_conv(i, 1, "act")
                    if i % GRP == GRP - 1:
                        g = i // GRP
                        dma_q.append((emit_stage_dma, (g, 1)))
                        dma_q.extend(
                            (emit_group_dma, (g, 1, q)) for q in range(NQ)
                        )
            if dma_q:
                fn, args = dma_q.pop(0)
                fn(*args)
            if s == 0:
                d0 = bigm
                ini = init0[:, 0:1]
            else:
                pb = new[(s - 1) % 4]
                mb = mm[s % 2]
                nc.vector.scalar_tensor_tensor(
                    out=mb, in0=pb[:, 1 : W + 1], scalar=0.0,
                    in1=pb[:, 0:W], op0=Alu.bypass, op1=Alu.min,
                )
                if s in (2, 4, 6):
                    q = s // 2
                    nc.vector.memset(mb[q * 32 : (q + 1) * 32, :], BIG)
                d0 = mb
                ini = new[s % 4][:, 0:1]
            nc.vector.tensor_tensor_scan(
                out=new[s % 4][:, 1 : W + 1],
                data0=d0,
                data1=csk[:, s * W : (s + 1) * W],
                initial=ini,
                op0=Alu.min,
                op1=Alu.add,
            )
            if s + 2 < NSS:
                tgt = new[(s + 2) % 4]
                nb = new[s % 4]
                nc.gpsimd.tensor_copy(out=tgt[32:64, 0:1], in_=nb[0:32, W : W + 1])
                nc.gpsimd.tensor_copy(out=tgt[64:96, 0:1], in_=nb[32:64, W : W + 1])
                nc.gpsimd.tensor_copy(out=tgt[96:128, 0:1], in_=nb[64:96, W : W + 1])
            if s >= R - 1 and (s - (R - 1)) % 2 == 0 and (s - (R - 1)) // 2 < NQ:
                q = (s - (R - 1)) // 2
                nc.scalar.copy(
                    out=outbuf[0:32, q * W : (q + 1) * W],
                    in_=new[s % 4][q * 32 : (q + 1) * 32, 1 : W + 1],
                )
        nc.sync.dma_start(out=out_d, in_=outbuf)


def _build():
    import concourse.bacc as bacc
    import concourse.tile as tile
    from concourse import mybir

    F32 = mybir.dt.float32
    BF16 = mybir.dt.bfloat16
    nc = bacc.Bacc()
    u_d = nc.dram_tensor("u_all", [KAUG, VB * R], BF16, kind="ExternalInput")[:]
    v_d = nc.dram_tensor("v_all", [KAUG, VB * L2], BF16, kind="ExternalInput")[:]
    out_d = nc.dram_tensor("out_rows", [VB, L2], F32, kind="ExternalOutput")[:]
    with tile.TileContext(nc) as tc:
        _emit(tc, u_d, v_d, out_d)
    nc.compile()
    return nc


def _host_prep(s1, s2):
    """Per-core u_all [18, 32*256] and v_all [18, 32*512] bf16 tensors."""
    import ml_dtypes

    BF = ml_dtypes.bfloat16
    s1 = np.ascontiguousarray(s1, dtype=np.float32)
    s2 = np.ascontiguousarray(s2, dtype=np.float32)
    in_maps = []
    for c in range(N_CORES):
        s1c = s1[c * PER_CORE : (c + 1) * PER_CORE]  # [16, 512, 16]
        s2c = s2[c * PER_CORE : (c + 1) * PER_CORE]
        s1v = np.concatenate([s1c[:, :R], s1c[:, ::-1][:, :R]], axis=0)  # [32,256,16]
        s2v = np.concatenate([s2c, s2c[:, ::-1]], axis=0)  # [32,512,16]
        u = np.empty((VB, R, KAUG), np.float32)
        u[:, :, :D] = -2.0 * s1v
        u[:, :, D] = 1.0
        u[:, :, D + 1] = (s1v * s1v).sum(-1)
        v = np.empty((VB, L2, KAUG), np.float32)
        v[:, :, :D] = s2v
        v[:, :, D] = (s2v * s2v).sum(-1)
        v[:, :, D + 1] = 1.0
        in_maps.append(
            {
                "u_all": np.ascontiguousarray(u.transpose(2, 0, 1).reshape(KAUG, VB * R)).astype(BF),
                "v_all": np.ascontiguousarray(v.transpose(2, 0, 1).reshape(KAUG, VB * L2)).astype(BF),
            }
        )
    return in_maps


def _combine(outs):
    """outs: list of [VB, 512] final-row arrays per core -> scalar loss."""
    vals = np.empty(B, np.float64)
    for c in range(N_CORES):
        rows = outs[c]
        for bl in range(PER_CORE):
            F = rows[bl].astype(np.float64)
            Brow = rows[PER_CORE + bl][::-1].astype(np.float64)
            Bnext = np.concatenate([Brow[1:], [np.inf]])
            vals[c * PER_CORE + bl] = np.min(F + np.minimum(Brow, Bnext))
    return np.float32(np.mean(np.sqrt(vals)))


def kernel(s1_batch, s2_batch):
    from concourse import bass_utils

    if "nc" not in _CACHE:
        _CACHE["nc"] = _build()
    nc = _CACHE["nc"]
    in_maps = _host_prep(np.asarray(s1_batch), np.asarray(s2_batch))
    kw = {}
    if _CACHE.get("trace"):
        kw = dict(trace=True, trace_cores=_CACHE.get("trace_cores", [0]),
                  tmpdir=_CACHE.get("tmpdir"))
    res = bass_utils.run_bass_kernel_spmd(
        nc, in_maps, core_ids=list(range(N_CORES)), **kw
    )
    if res.exec_time_ns is not None:
        _CACHE["exec_time_ns"] = res.exec_time_ns
    _CACHE["last_results"] = res
    outs = [r["out_rows"] for r in res.results]
    return _combine(outs)


# revision 18
# speedup vs baseline: 1.5510x; 1.1373x over previous
"""DTW loss kernel for Trainium2 (8 NeuronCores, Bass/Tile) — v2.

reference: C[b,i,j] = ||s1[b,i]-s2[b,j]||^2 ; DTW DP over [512,512]; return
mean_b sqrt(DTW[b,-1,-1]).

Meet-in-the-middle: DTW_end = min_j F[255,j] + min(B[256,j], B[256,j+1]) where
F is the forward DP over rows 0..255 and B the backward DP. Each core: 16
batch elems x 2 directions = 32 virtual batches (vb) of 256-row half-DPs.

v2 design (vs v1's per-superstep psum gather copies):
- Cost rows come from per-vb matmuls: lhsT = u[vb] chunk [18,128],
  rhs = v[vb] [18,512] -> psum [128 rows, 512 cols] f32. 64 matmuls total
  (5x fewer PE cycles than v1's block-diagonal batching, whose cost was
  dominated by 5 K-chunk passes over the 512-wide output).
- Act/DVE convert psum -> fp16 staging [rows, 512].
- Skewed cost buffer csk[q*32+vb, s*W + j] = C[vb, s-2q, q*W+j] in SBUF.
  A single SBUF->SBUF DMA cannot move data across the partition-dim meaning
  (SBUF APs are partition-major), so the reshape takes a DRAM round-trip:
  stage -> dram c_rt[vb, r, j] per vb, then dram -> csk per (band, vb-group)
  with matching flat iteration orders. The DP scan then reads data1 directly
  from csk with zero per-superstep copies (v1 burned the Scalar engine on 3
  gather copies per superstep).
- Wavefront: NQ=4 bands of W=128 cols on 128 partitions = (band q, vb).
  Superstep s: band q scans row s-2q. Carries between bands ride in col 0
  of the `new` tiles (one [32,1] partition-shifted copy per band boundary,
  2 supersteps of slack, on Act/Pool engines).
"""

import numpy as np

B = 128
L1 = 512
L2 = 512
D = 16
N_CORES = 8
PER_CORE = B // N_CORES  # 16
VB = 2 * PER_CORE  # 32
R = L1 // 2  # 256
KAUG = D + 2  # 18
NQ = 4
W = L2 // NQ  # 128
NSS = R + 2 * (NQ - 1)  # 262
GRP = 8  # vb per staging group
NGRP = VB // GRP  # 4
BIG = 1e30

_CACHE = {}


def _emit(tc, u_d, v_d, out_d):
    import concourse.bass as bass  # noqa: F401
    from concourse import mybir

    F32 = mybir.dt.float32
    F16 = mybir.dt.float16
    BF16 = mybir.dt.bfloat16
    Alu = mybir.AluOpType
    nc = tc.nc

    with (
        tc.tile_pool(name="singles", bufs=1) as sp,
        tc.tile_pool(name="psum", bufs=4, space="PSUM") as pp,
        tc.tile_pool(name="dramp", bufs=1, space="DRAM") as dp,
    ):
        # band-major DRAM scratch: c_rt2[vb, q, r, j] = C[vb, r, q*W+j].
        # DMA1 (stage->dram) pays the 256B-strided side as posted writes;
        # DMA2 (dram->csk) then reads contiguous 32KB runs per (vb,q).
        c_rt = dp.tile([VB, NQ, R, W], mybir.dt.float16, tag="crt", name="crt")
        u_sb = sp.tile([KAUG, VB * R], BF16, tag="u", name="u")
        v_sb = sp.tile([KAUG, VB * L2], BF16, tag="v", name="v")
        csk = sp.tile([128, NSS * W], F16, tag="csk", name="csk")
        stage = [
            sp.tile([128, GRP * L2], F16, tag=f"stg{i}", name=f"stg{i}")
            for i in range(4)
        ]
        new = [sp.tile([128, W + 1], F32, tag=f"new{p}", name=f"new{p}") for p in range(4)]
        mm = [sp.tile([128, W], F32, tag=f"m{p}", name=f"m{p}") for p in range(2)]
        bigm = sp.tile([128, W], F32, tag="bigm", name="bigm")
        init0 = sp.tile([128, 1], F32, tag="init0", name="init0")
        outbuf = sp.tile([32, L2], F32, tag="ob", name="ob")

        # --- prologue ---
        nc.sync.dma_start(out=u_sb, in_=u_d)
        nc.sync.dma_start(out=v_sb, in_=v_d)
        nc.vector.memset(bigm, BIG)
        nc.vector.memset(init0, 0.0)
        for p in range(4):
            nc.vector.memset(new[p][:, 0:1], BIG)
        # csk margins (supersteps where a band has no valid row)
        nc.vector.memset(csk[:, 0 : 2 * (NQ - 1) * W], 0.0)
        nc.vector.memset(csk[:, R * W : NSS * W], 0.0)

        def emit_mm_conv(vb, h, eng):
            pt = pp.tile([128, L2], F32, tag="pt", name=f"pt{vb}_{h}")
            nc.tensor.matmul(
                out=pt,
                lhsT=u_sb[:, vb * R + h * 128 : vb * R + h * 128 + 128],
                rhs=v_sb[:, vb * L2 : (vb + 1) * L2],
                start=True,
                stop=True,
            )
            g = vb // GRP
            dst = stage[g % 4][:, (vb % GRP) * L2 : (vb % GRP + 1) * L2]
            if eng == "act":
                nc.scalar.copy(out=dst, in_=pt)
            else:
                nc.vector.tensor_copy(out=dst, in_=pt)

        RQ = 32  # rows per reshape-DMA chunk (finer => earlier DP start)

        def emit_stage_dma(g, h, rc):
            # stage [r-part, (vb,q,j)] -> c_rt2[vb, q, 128h+r, j]: iteration
            # (r, (vb,q), j); out (vb,q) merges to one uniform-stride dim.
            # SWDGE (gpsimd) so the 256B descriptors cost 0.34ns each.
            buf = stage[g % 4]
            r0 = 128 * h + rc * RQ
            out_ap = c_rt[g * GRP : (g + 1) * GRP, :, r0 : r0 + RQ, :]
            out_ap = out_ap.rearrange("v q r j -> r (v q) j")
            in_ap = buf[rc * RQ : (rc + 1) * RQ, :].rearrange(
                "r (vq j) -> r vq j", j=W
            )
            nc.gpsimd.dma_start(out=out_ap, in_=in_ap)

        def emit_group_dma(g, h, q, rc):
            # dram -> csk: contiguous runs per vb on the dram side
            r0 = 128 * h + rc * RQ
            in_ap = c_rt[g * GRP : (g + 1) * GRP, q, r0 : r0 + RQ, :]
            base = (r0 + 2 * q) * W
            out_ap = csk[
                q * 32 + g * GRP : q * 32 + g * GRP + GRP,
                base : base + RQ * W,
            ].rearrange("v (r j) -> v r j", r=RQ)
            nc.sync.dma_start(out=out_ap, in_=in_ap)

        # --- h=0 production (pipeline fill) ---
        # emit all converts first, then reshape DMAs in row-chunk-major order
        # so the first RQ rows of every group land before later rows
        for vb in range(VB):
            emit_mm_conv(vb, 0, "dve" if vb % 2 else "act")
        for rc in range(128 // RQ):
            for g in range(NGRP):
                emit_stage_dma(g, 0, rc)
                for q in range(NQ):
                    emit_group_dma(g, 0, q, rc)

        # --- DP loop with h=1 production interleaved ---
        # pacing: one matmul+convert every 3 supersteps; a finished group's
        # stage->dram DMA then its 4 dram->csk DMAs, one per superstep
        dma_q = []  # pending emitters
        for s in range(NSS):
            if s >= 4 and (s - 4) % 3 == 0:
                i = (s - 4) // 3
                if i < VB:
                    emit_mm_conv(i, 1, "act")
                    if i % GRP == GRP - 1:
                        g = i // GRP
                        for rc in range(128 // RQ):
                            dma_q.append((emit_stage_dma, (g, 1, rc)))
                            dma_q.extend(
                                (emit_group_dma, (g, 1, q, rc)) for q in range(NQ)
                            )
            if dma_q:
                fn, args = dma_q.pop(0)
                fn(*args)
            if s == 0:
                d0 = bigm
                ini = init0[:, 0:1]
            else:
                pb = new[(s - 1) % 4]
                mb = mm[s % 2]
                nc.vector.scalar_tensor_tensor(
                    out=mb, in0=pb[:, 1 : W + 1], scalar=0.0,
                    in1=pb[:, 0:W], op0=Alu.bypass, op1=Alu.min,
                )
                if s in (2, 4, 6):
                    q = s // 2
                    nc.vector.memset(mb[q * 32 : (q + 1) * 32, :], BIG)
                d0 = mb
                ini = new[s % 4][:, 0:1]
            nc.vector.tensor_tensor_scan(
                out=new[s % 4][:, 1 : W + 1],
                data0=d0,
                data1=csk[:, s * W : (s + 1) * W],
                initial=ini,
                op0=Alu.min,
                op1=Alu.add,
            )
            if s + 2 < NSS:
                tgt = new[(s + 2) % 4]
                nb = new[s % 4]
                nc.gpsimd.tensor_copy(out=tgt[32:64, 0:1], in_=nb[0:32, W : W + 1])
                nc.gpsimd.tensor_copy(out=tgt[64:96, 0:1], in_=nb[32:64, W : W + 1])
                nc.gpsimd.tensor_copy(out=tgt[96:128, 0:1], in_=nb[64:96, W : W + 1])
            if s >= R - 1 and (s - (R - 1)) % 2 == 0 and (s - (R - 1)) // 2 < NQ:
                q = (s - (R - 1)) // 2
                nc.scalar.copy(
                    out=outbuf[0:32, q * W : (q + 1) * W],
                    in_=new[s % 4][q * 32 : (q + 1) * 32, 1 : W + 1],
                )
        nc.sync.dma_start(out=out_d, in_=outbuf)


def _build():
    import concourse.bacc as bacc
    import concourse.tile as tile
    from concourse import mybir

    F32 = mybir.dt.float32
    BF16 = mybir.dt.bfloat16
    nc = bacc.Bacc()
    u_d = nc.dram_tensor("u_all", [KAUG, VB * R], BF16, kind="ExternalInput")[:]
    v_d = nc.dram_tensor("v_all", [KAUG, VB * L2], BF16, kind="ExternalInput")[:]
    out_d = nc.dram_tensor("out_rows", [VB, L2], F32, kind="ExternalOutput")[:]
    with tile.TileContext(nc) as tc:
        _emit(tc, u_d, v_d, out_d)
    nc.compile()
    return nc


def _host_prep(s1, s2):
    """Per-core u_all [18, 32*256] and v_all [18, 32*512] bf16 tensors."""
    import ml_dtypes

    BF = ml_dtypes.bfloat16
    s1 = np.ascontiguousarray(s1, dtype=np.float32)
    s2 = np.ascontiguousarray(s2, dtype=np.float32)
    in_maps = []
    for c in range(N_CORES):
        s1c = s1[c * PER_CORE : (c + 1) * PER_CORE]  # [16, 512, 16]
        s2c = s2[c * PER_CORE : (c + 1) * PER_CORE]
        s1v = np.concatenate([s1c[:, :R], s1c[:, ::-1][:, :R]], axis=0)  # [32,256,16]
        s2v = np.concatenate([s2c, s2c[:, ::-1]], axis=0)  # [32,512,16]
        u = np.empty((VB, R, KAUG), np.float32)
        u[:, :, :D] = -2.0 * s1v
        u[:, :, D] = 1.0
        u[:, :, D + 1] = (s1v * s1v).sum(-1)
        v = np.empty((VB, L2, KAUG), np.float32)
        v[:, :, :D] = s2v
        v[:, :, D] = (s2v * s2v).sum(-1)
        v[:, :, D + 1] = 1.0
        in_maps.append(
            {
                "u_all": np.ascontiguousarray(u.transpose(2, 0, 1).reshape(KAUG, VB * R)).astype(BF),
                "v_all": np.ascontiguousarray(v.transpose(2, 0, 1).reshape(KAUG, VB * L2)).astype(BF),
            }
        )
    return in_maps


def _combine(outs):
    """outs: list of [VB, 512] final-row arrays per core -> scalar loss."""
    vals = np.empty(B, np.float64)
    for c in range(N_CORES):
        rows = outs[c]
        for bl in range(PER_CORE):
            F = rows[bl].astype(np.float64)
            Brow = rows[PER_CORE + bl][::-1].astype(np.float64)
            Bnext = np.concatenate([Brow[1:], [np.inf]])
            vals[c * PER_CORE + bl] = np.min(F + np.minimum(Brow, Bnext))
    return np.float32(np.mean(np.sqrt(vals)))


def kernel(s1_batch, s2_batch):
    from concourse import bass_utils

    if "nc" not in _CACHE:
        _CACHE["nc"] = _build()
    nc = _CACHE["nc"]
    in_maps = _host_prep(np.asarray(s1_batch), np.asarray(s2_batch))
    kw = {}
    if _CACHE.get("trace"):
        kw = dict(trace=True, trace_cores=_CACHE.get("trace_cores", [0]),
                  tmpdir=_CACHE.get("tmpdir"))
    res = bass_utils.run_bass_kernel_spmd(
        nc, in_maps, core_ids=list(range(N_CORES)), **kw
    )
    if res.exec_time_ns is not None:
        _CACHE["exec_time_ns"] = res.exec_time_ns
    _CACHE["last_results"] = res
    outs = [r["out_rows"] for r in res.results]
    return _combine(outs)
